# revision 3
# baseline (speedup 1.0000x reference)
"""Trainium2 Bass kernel for the N-gram language model problem.

Math:  logits[b,t,o] = sum_kv emb[idx[b,t+k], v] * W[o, k*V+v] + b[o]
       (B=8, T=1024, N=2, V=8192)

Key restructuring: precompute fused tables
    M0 = emb @ W0^T   (V x V),  W0 = W[:, :V]
    M1 = emb @ W1^T + b
then  logits[b,t] = M0[idx[b,t]] + M1[idx[b,t+1]].

Sharding: tensor-parallel over the output vocab dim o across 8 cores
(1024 columns each).  Per core:
  phase 1: M0_s (V x 1024), M1_s (V x 1024) via one tiled f32r matmul
           (contraction over v, k-tiles of 128, PSUM accumulation)
  phase 2: indirect-DMA row gathers M0_s[idx0], M1_s[idx1] + vector add.
"""

import os
import sys

import numpy as np

for _p in ("/opt/trn_rl_repo", "/root/.axon_site/_ro/trn_rl_repo"):
    if os.path.isdir(_p) and _p not in sys.path:
        sys.path.append(_p)

import concourse.bacc as bacc
import concourse.bass as bass
import concourse.mybir as mybir
import concourse.tile as tile
from concourse import bass_utils

# ---- problem constants (full config) ----
V = 8192        # vocab
NGRAM = 2       # context length
B = 8
T = 1024
NCORES = 8
TOK = B * (T - NGRAM)   # 8176 output rows
OS = 2 * V // NCORES // 2  # = 1024 output columns per core (each of M0/M1)

P = 128         # partitions


def _ceil_to(x, m):
    return (x + m - 1) // m * m


def build_nc(V_=V, OS_=OS, TOKP_=None, IG=4, KCH=4, NBW=512):
    """Build the per-core Bass kernel.

    V_   : vocab / contraction size (k = v dim), multiple of 128
    OS_  : per-core output-column shard width for each of M0/M1
    TOKP_: padded token count (multiple of 128)
    IG   : i-tiles (of 128 rows) per slab group
    KCH  : k-tiles (of 128) per wt DMA chunk
    NBW  : matmul free-dim block width (<= 512)
    """
    if TOKP_ is None:
        TOKP_ = _ceil_to(TOK, P)
    f32 = mybir.dt.float32
    f32r = mybir.dt.float32r
    i32 = mybir.dt.int32

    n_ktile = V_ // P
    n_itile = V_ // P
    n_group = (n_itile + IG - 1) // IG
    NB = 2 * OS_ // NBW            # column blocks over [M0 | M1]
    nb_per_tab = OS_ // NBW
    n_kchunk = (n_ktile + KCH - 1) // KCH
    n_ttile = TOKP_ // P

    nc = bacc.Bacc(None, target_bir_lowering=False)

    embT = nc.dram_tensor("embT", (V_, V_), f32r, kind="ExternalInput")
    wt = nc.dram_tensor("wt", (V_, 2 * OS_), f32r, kind="ExternalInput")
    idx0 = nc.dram_tensor("idx0", (TOKP_, 1), i32, kind="ExternalInput")
    idx1 = nc.dram_tensor("idx1", (TOKP_, 1), i32, kind="ExternalInput")
    biasb = nc.dram_tensor("biasb", (P, OS_), f32, kind="ExternalInput")
    out = nc.dram_tensor("out", (TOKP_, OS_), f32, kind="ExternalOutput")
    M0 = nc.dram_tensor("M0", (V_, OS_), f32)
    M1 = nc.dram_tensor("M1", (V_, OS_), f32)

    with tile.TileContext(nc) as tc:
        # ---------- phase 1: M0/M1 = embT.T @ wt ----------
        with tc.tile_pool(name="slab", bufs=1) as slab_pool, \
             tc.tile_pool(name="wtp", bufs=4) as wt_pool, \
             tc.tile_pool(name="psum", bufs=8, space="PSUM") as psum_pool, \
             tc.tile_pool(name="stage", bufs=4) as stage_pool, \
             tc.tile_pool(name="biasp", bufs=1) as bias_pool:

            bias_sb = bias_pool.tile([P, OS_], f32)
            nc.sync.dma_start(out=bias_sb[:], in_=biasb[:, :])

            for g in range(n_group):
                ig = min(IG, n_itile - g * IG)       # i-tiles in this group
                iw = ig * P                          # i columns in slab
                c0 = g * IG * P
                slab = slab_pool.tile([P, n_ktile * IG * P], f32r, tag="slab")
                for kc in range(n_kchunk):
                    kt0 = kc * KCH
                    kn = min(KCH, n_ktile - kt0)
                    src = embT[kt0 * P:(kt0 + kn) * P, c0:c0 + iw]
                    nc.sync.dma_start(
                        out=slab[:, kt0 * iw:(kt0 + kn) * iw].rearrange(
                            "p (a i) -> p a i", a=kn),
                        in_=src.rearrange("(a p) i -> p a i", p=P),
                    )
                for nb in range(NB):
                    psums = [
                        psum_pool.tile([P, NBW], f32, tag="ps", name=f"ps_{g}_{nb}_{it}")
                        for it in range(ig)
                    ]
                    for kc in range(n_kchunk):
                        kt0 = kc * KCH
                        kn = min(KCH, n_ktile - kt0)
                        wtt = wt_pool.tile([P, KCH * NBW], f32r, tag="wtt",
                                           name=f"wtt_{g}_{nb}_{kc}")
                        src = wt[kt0 * P:(kt0 + kn) * P, nb * NBW:(nb + 1) * NBW]
                        nc.sync.dma_start(
                            out=wtt[:, :kn * NBW].rearrange(
                                "p (a o) -> p a o", a=kn),
                            in_=src.rearrange("(a p) o -> p a o", p=P),
                        )
                        for kl in range(kn):
                            kt = kt0 + kl
                            rhs = wtt[:, kl * NBW:(kl + 1) * NBW]
                            for it in range(ig):
                                nc.tensor.matmul(
                                    psums[it][:],
                                    lhsT=slab[:, kt * iw + it * P:kt * iw + (it + 1) * P],
                                    rhs=rhs,
                                    start=(kt == 0),
                                    stop=(kt == n_ktile - 1),
                                )
                    dstM = M0 if nb < nb_per_tab else M1
                    cb = (nb % nb_per_tab) * NBW
                    for it in range(ig):
                        st = stage_pool.tile([P, NBW], f32, tag="st",
                                             name=f"st_{g}_{nb}_{it}")
                        if nb < nb_per_tab:
                            nc.vector.tensor_copy(st[:], psums[it][:])
                        else:
                            nc.vector.tensor_add(st[:], psums[it][:],
                                                 bias_sb[:, cb:cb + NBW])
                        nc.sync.dma_start(
                            out=dstM[c0 + it * P:c0 + (it + 1) * P, cb:cb + NBW],
                            in_=st[:],
                        )

        # ---------- phase 2: out[t] = M0[idx0[t]] + M1[idx1[t]] ----------
        with tc.tile_pool(name="idxp", bufs=6) as idx_pool, \
             tc.tile_pool(name="gat", bufs=6) as gat_pool, \
             tc.tile_pool(name="outp", bufs=4) as out_pool:
            for tt in range(n_ttile):
                r0 = tt * P
                i0 = idx_pool.tile([P, 1], i32, tag="i0", name=f"i0_{tt}")
                nc.sync.dma_start(out=i0[:], in_=idx0[r0:r0 + P, :])
                i1 = idx_pool.tile([P, 1], i32, tag="i1", name=f"i1_{tt}")
                nc.sync.dma_start(out=i1[:], in_=idx1[r0:r0 + P, :])
                g0 = gat_pool.tile([P, OS_], f32, tag="g0", name=f"g0_{tt}")
                nc.gpsimd.indirect_dma_start(
                    out=g0[:], out_offset=None, in_=M0[:, :],
                    in_offset=bass.IndirectOffsetOnAxis(ap=i0[:, :1], axis=0),
                )
                g1 = gat_pool.tile([P, OS_], f32, tag="g1", name=f"g1_{tt}")
                nc.gpsimd.indirect_dma_start(
                    out=g1[:], out_offset=None, in_=M1[:, :],
                    in_offset=bass.IndirectOffsetOnAxis(ap=i1[:, :1], axis=0),
                )
                ot = out_pool.tile([P, OS_], f32, tag="ot", name=f"ot_{tt}")
                nc.vector.tensor_add(ot[:], g0[:], g1[:])
                nc.sync.dma_start(out=out[r0:r0 + P, :], in_=ot[:])

    nc.compile()
    return nc


def host_prep(idx, emb_table, W, b, V_=V, OS_=OS, TOKP_=None):
    """Build the 8 per-core input maps from the full problem inputs."""
    if TOKP_ is None:
        TOKP_ = _ceil_to(TOK, P)
    Bq, Tq = idx.shape
    Tn = Tq - NGRAM
    idx = np.asarray(idx)
    i0 = np.ascontiguousarray(idx[:, 0:Tn].reshape(-1).astype(np.int32))
    i1 = np.ascontiguousarray(idx[:, 1:Tn + 1].reshape(-1).astype(np.int32))
    ntok = i0.shape[0]
    i0p = np.zeros((TOKP_, 1), np.int32)
    i1p = np.zeros((TOKP_, 1), np.int32)
    i0p[:ntok, 0] = i0
    i1p[:ntok, 0] = i1

    embT = np.ascontiguousarray(np.asarray(emb_table, np.float32).T)
    W = np.asarray(W, np.float32)
    b = np.asarray(b, np.float32)

    in_maps = []
    for c in range(NCORES):
        sh = slice(c * OS_, (c + 1) * OS_)
        wt_c = np.concatenate(
            [np.ascontiguousarray(W[sh, :V_].T),
             np.ascontiguousarray(W[sh, V_:].T)], axis=1)
        bias_c = np.broadcast_to(b[sh][None, :], (P, OS_)).copy()
        in_maps.append({
            "embT": embT,
            "wt": wt_c,
            "idx0": i0p,
            "idx1": i1p,
            "biasb": bias_c,
        })
    return in_maps, ntok


_NC_CACHE = {}


def kernel(idx, emb_table, W, b, _trace=False):
    key = "full"
    if key not in _NC_CACHE:
        _NC_CACHE[key] = build_nc()
    nc = _NC_CACHE[key]

    in_maps, ntok = host_prep(idx, emb_table, W, b)
    res = bass_utils.run_bass_kernel_spmd(
        nc, in_maps, core_ids=list(range(NCORES)), trace=_trace)

    Bq, Tq = np.asarray(idx).shape
    Tn = Tq - NGRAM
    full = np.empty((ntok, V), np.float32)
    for c in range(NCORES):
        full[:, c * OS:(c + 1) * OS] = res.results[c]["out"][:ntok]
    out = full.reshape(Bq, Tn, V)
    if _trace:
        return out, res
    return out
